# revision 24
# baseline (speedup 1.0000x reference)
"""Trainium2 Bass kernel for nn_Conv2dModulated (modulated transposed conv + blur).

Math restructure (validated vs reference to 5e-7 rel in fp32):
  s = w @ affine_w.T + affine_b + 1                    (B, CIN)  host
  d = rsqrt(s^2 @ sum_kk(W^2).T + 1e-8)               (B, COUT) host
  out[b] = d[b,:]/16 * blur(convT2x(s[b,:] * x[b], W)) + bias
- Modulation folds into x (per-input-channel scale), demodulation into the
  PSUM eviction (per-output-channel scale) -> weights stay sample-independent.
- Stride-2 transposed conv = 4 parity classes of <=2x2-tap convs on the 32x32
  input (subpixel decomposition; 9 effective taps instead of 36 dilated).
- The 4x4 blur ([1,3,3,1] x [1,3,3,1])/16 = three [1,1] adds per dim in bf16,
  split between DVE and the Pool (GPSIMD) engine; /16 folded into d.
- Output rows are processed in 3 bands so eviction/blur/store pipeline at
  band granularity against the PE matmul stream (short tail), and each
  weight tile is loaded once per band and reused for both samples.
- Output stored as bf16; upcast + bias on host.

Sharding: data-parallel over batch, 2 samples per core, 8 cores, no
collectives.
"""

import os
from contextlib import ExitStack

import numpy as np
import ml_dtypes

import concourse.bass as bass
import concourse.tile as tile
from concourse import mybir
from concourse.bass_utils import run_bass_kernel_spmd

B, CIN, COUT, LAT, H, W_SP, KK = 16, 512, 512, 512, 32, 32, 3
NCORES = 8
BPC = B // NCORES  # samples per core
P = 128
NCI = CIN // P
NCO = COUT // P
BF16 = mybir.dt.bfloat16
F32 = mybir.dt.float32
ADD = mybir.AluOpType.add

# parity classes: (eh, ec, rtaps, ctaps, ncols); taps are (k_index, offset)
CLASSES = [
    (0, 0, [(0, 0), (2, 1)], [(0, 0), (2, 1)], 33),
    (0, 1, [(0, 0), (2, 1)], [(1, 1)], 32),
    (1, 0, [(1, 1)], [(0, 0), (2, 1)], 33),
    (1, 1, [(1, 1)], [(1, 1)], 32),
]
# flat tap index (kh*3+kw) lists per class, defining the class-grouped
# weight layout used for input DMA
CLASS_TAPS = [[0, 2, 6, 8], [1, 7], [3, 5], [4]]
CLASS_OFF = [0, 4, 6, 8]  # tap-slot offset of each class block
# row bands over the class-local output rows (u coordinate), per oc-tile.
# The last oc uses fine bands so its post-PE blur work drains incrementally
# (short tail); earlier ocs use coarse bands (fewer instructions).
OC_U = {0: [0, 14, 28, 33], 1: [0, 14, 28, 33], 2: [0, 14, 28, 33],
        3: [0, 7, 14, 21, 28, 33]}


def band_tables(U):
    """Derive per-band row ranges: matmul u-bands per row parity, plus the
    row ranges of each blur stage (padded y: 67 rows -> zz/c1/c2/of)."""
    nb = len(U) - 1
    bands = {0: [], 1: []}
    for i in range(nb):
        u0, u1 = U[i], U[i + 1]
        bands[0].append((u0, u1 - u0))
        bands[1].append((u0, min(u1, 32) - u0))
    zzE = [2 * U[i + 1] + 1 for i in range(nb)]
    zzE[-1] = 67
    c1E = [z - 1 for z in zzE]
    c2E = [z - 1 for z in c1E]
    ofE = [z - 1 for z in c2E]

    def rng(E):
        return [
            ((0 if i == 0 else E[i - 1]), E[i] - (0 if i == 0 else E[i - 1]))
            for i in range(nb)
        ]

    colb = rng(zzE)
    return bands, colb, rng(c1E), rng(c2E), rng(ofE)


OC_TABLES = {oc: band_tables(U) for oc, U in OC_U.items()}

_ENG_PREFIX = {
    "PE": "PE_", "DVE": "DVE_", "Activation": "Activation_",
    "Pool": "Pool_", "SP": "SP_",
}


def _fix_waits(nc: bass.Bass) -> None:
    """Walrus codegen accepts only one sem-wait per compute instruction;
    Tile emits up to 4.

    1) Drop same-engine self-waits: every engine executes its stream
       serially in order (PE matmul completion is pc-monotone; DVE/ACT
       have a hardware output-drain between ops), so a wait on the
       engine's own completion semaphore is redundant.
    2) Split any remaining multi-wait onto same-engine NoOp instructions
       inserted just before the instruction.
    """
    for f in nc.m.functions:
        for bb in f.blocks:
            out = []
            for inst in bb.instructions:
                si = inst.sync_info
                if si is None or len(si.on_wait) <= 1:
                    out.append(inst)
                    continue
                eng = str(inst.engine).split(".")[-1]
                pfx = _ENG_PREFIX.get(eng)
                waits = list(si.on_wait)
                keep = [
                    w for w in waits
                    if not (pfx and (w.ant_name or "").startswith(pfx))
                ]
                for w in keep[:-1]:
                    nop = mybir.InstNoOp(name=nc.get_next_instruction_name())
                    nop.engine = inst.engine
                    nop.sync_info = mybir.SyncInfo(on_wait=[w], on_update=[])
                    out.append(nop)
                keep = keep[-1:]
                inst.sync_info = mybir.SyncInfo(
                    on_wait=keep, on_update=list(si.on_update)
                )
                out.append(inst)
            bb.instructions = out


def build_program() -> bass.Bass:
    nc = bass.Bass()
    xp_d = nc.declare_dram_parameter("xp", [BPC, NCI, P, 34 * 34], BF16, isOutput=False)
    wt_d = nc.declare_dram_parameter("wt", [NCI, P, 9 * COUT], BF16, isOutput=False)
    dsc_d = nc.declare_dram_parameter("dsc", [P, BPC * NCO], F32, isOutput=False)
    out_d = nc.declare_dram_parameter("out", [BPC, NCO, P, 64 * 64], BF16, isOutput=True)

    with ExitStack() as ctx:
        tc = ctx.enter_context(tile.TileContext(nc))
        consts = ctx.enter_context(tc.tile_pool(name="consts", bufs=1))
        psum = ctx.enter_context(tc.tile_pool(name="psum", bufs=8, space="PSUM"))
        spool = ctx.enter_context(tc.tile_pool(name="spool", bufs=4))
        zpool = ctx.enter_context(tc.tile_pool(name="zpool", bufs=3))
        c1pool = ctx.enter_context(tc.tile_pool(name="c1pool", bufs=2))
        c2pool = ctx.enter_context(tc.tile_pool(name="c2pool", bufs=2))
        opool = ctx.enter_context(tc.tile_pool(name="opool", bufs=4))

        # --- input DMAs, round-robin over the HWDGE queues in first-use order
        queues = [nc.sync, nc.scalar]
        qi = [0]

        def q():
            e = queues[qi[0] % len(queues)]
            qi[0] += 1
            return e

        d_sb = consts.tile([P, BPC * NCO], F32, tag="dsb")
        w_sb = {}
        for c in range(NCI):
            w_sb[c] = consts.tile([P, 9 * COUT], BF16, tag=f"w{c}", name=f"w{c}")
        x_tiles = {}
        for s in range(BPC):
            for c in range(NCI):
                x_tiles[(s, c)] = consts.tile(
                    [P, 34, 34], BF16, tag=f"x{s}{c}", name=f"x{s}{c}"
                )

        # class-grouped weight chunks, interleaved with x tiles in first-use
        # order across both HWDGE queues
        def dma_w(ci, c, ti=None):
            lo = CLASS_OFF[ci] * COUT
            hi = lo + len(CLASS_TAPS[ci]) * COUT
            if ti is not None:
                lo, hi = lo + ti * COUT, lo + (ti + 1) * COUT
            q().dma_start(out=w_sb[c][:, lo:hi], in_=wt_d[c][:, lo:hi])

        def dma_x(s, c):
            q().dma_start(
                out=x_tiles[(s, c)][:],
                in_=xp_d[s, c].rearrange("p (a b) -> p a b", b=34),
            )

        dma_x(0, 0)
        dma_x(1, 0)
        for ti in range(4):  # first cin's class00 weights, per-tap chunks
            dma_w(0, 0, ti)
        dma_x(0, 1)
        dma_x(1, 1)
        dma_w(0, 1)
        dma_x(0, 2)
        dma_x(1, 2)
        dma_w(0, 2)
        dma_x(0, 3)
        dma_x(1, 3)
        dma_w(0, 3)
        q().dma_start(out=d_sb[:], in_=dsc_d[:])
        for ci in (1, 2, 3):
            for c in range(NCI):
                dma_w(ci, c)

        # persistent padded y buffers; zeroed once on ACT (borders stay zero,
        # interior fully overwritten every reuse)
        y_tiles = []
        for i in range(3):
            yt = consts.tile([P, 67 * 67 + 1], BF16, tag=f"ybuf{i}")
            nc.scalar.memzero(yt[:])
            y_tiles.append(yt)

        def unit_idx(s, oc):
            return oc * BPC + s

        y65ps = {}
        zzbufs = {}
        c1bufs = {}
        c2bufs = {}

        def policy(oc, b, s, stage):
            """engine per blur stage; Pool takes ~45% of the element work.
            The very last band is sample-split so the tail drains on both
            engines in parallel."""
            nb = len(OC_TABLES[oc][1])
            if oc == NCO - 1 and b == nb - 1:
                return nc.gpsimd if s == 1 else nc.vector
            if stage in ("s1", "c1"):
                return nc.gpsimd
            if stage == "c2" and s == 0:
                return nc.gpsimd
            return nc.vector

        def emit_cols(s, oc, b):
            """column blur passes for band b of unit (s, oc)"""
            r0, nrow = OC_TABLES[oc][1][b]
            y65p = y65ps[(s, oc)]
            s1 = spool.tile([P, nrow, 66], BF16, tag="scr1")
            policy(oc, b, s, "s1").tensor_add(
                s1[:], y65p[:, r0 : r0 + nrow, 0:66], y65p[:, r0 : r0 + nrow, 1:67]
            )
            s2 = spool.tile([P, nrow, 65], BF16, tag="scr2")
            policy(oc, b, s, "s2").tensor_add(s2[:], s1[:, :, 0:65], s1[:, :, 1:66])
            zz = zzbufs[(s, oc)]
            policy(oc, b, s, "zz").tensor_add(
                zz[:, r0 : r0 + nrow, :], s2[:, :, 0:64], s2[:, :, 1:65]
            )

        def emit_rows(s, oc, b):
            """row blur passes + store for band b of unit (s, oc)"""
            _, _, C1B, C2B, OFB = OC_TABLES[oc]
            zz = zzbufs[(s, oc)]
            c1 = c1bufs[(s, oc)]
            c2 = c2bufs[(s, oc)]
            r0, nr = C1B[b]
            policy(oc, b, s, "c1").tensor_add(
                c1[:, r0 : r0 + nr, :],
                zz[:, r0 : r0 + nr, :],
                zz[:, r0 + 1 : r0 + nr + 1, :],
            )
            r0, nr = C2B[b]
            policy(oc, b, s, "c2").tensor_add(
                c2[:, r0 : r0 + nr, :],
                c1[:, r0 : r0 + nr, :],
                c1[:, r0 + 1 : r0 + nr + 1, :],
            )
            r0, nr = OFB[b]
            of = opool.tile([P, nr, 64], BF16, tag="out")
            policy(oc, b, s, "of").tensor_add(
                of[:], c2[:, r0 : r0 + nr, :], c2[:, r0 + 1 : r0 + nr + 1, :]
            )
            oq = nc.scalar if (oc == NCO - 1 and (b + s) % 2 == 0) else nc.sync
            oq.dma_start(
                out=out_d[s, oc][:, r0 * 64 : (r0 + nr) * 64],
                in_=of[:].rearrange("p a b -> p (a b)"),
            )

        for oc in range(NCO):
            for s in range(BPC):
                u = unit_idx(s, oc)
                y65ps[(s, oc)] = y_tiles[u % 3][:, 0 : 67 * 67].rearrange(
                    "p (a b) -> p a b", b=67
                )
                zzbufs[(s, oc)] = zpool.tile([P, 67, 64], BF16, tag="zz", name=f"zz{u}")
                c1bufs[(s, oc)] = c1pool.tile([P, 66, 64], BF16, tag="c1", name=f"c1{u}")
                c2bufs[(s, oc)] = c2pool.tile([P, 65, 64], BF16, tag="c2", name=f"c2{u}")
            BANDS = OC_TABLES[oc][0]
            nbands = len(BANDS[0])
            for b in range(nbands):
                for ci, (eh, ec, rtaps, ctaps, ncols) in enumerate(CLASSES):
                    u0, nr = BANDS[eh][b]
                    taps = [
                        (kh, kw, ra, cb) for (kh, ra) in rtaps for (kw, cb) in ctaps
                    ]
                    pts = [
                        psum.tile([P, 512], F32, tag="ps", name=f"ps{oc}{b}{eh}{ec}{s}")
                        for s in range(BPC)
                    ]
                    nmm = len(taps) * NCI
                    i = 0
                    for c in range(NCI):
                        for ti, (kh, kw, ra, cb) in enumerate(taps):
                            toff = (CLASS_OFF[ci] + ti) * COUT + oc * P
                            lhsT = w_sb[c][:, toff : toff + P]
                            for s in range(BPC):
                                rhs = x_tiles[(s, c)][
                                    :, u0 + ra : u0 + ra + nr, cb : cb + ncols
                                ]
                                nc.tensor.matmul(
                                    pts[s][:, : nr * ncols], lhsT, rhs,
                                    start=(i == 0), stop=(i == nmm - 1),
                                )
                            i += 1
                    for s in range(BPC):
                        src = pts[s][:, : nr * ncols].rearrange(
                            "p (r c) -> p r c", c=ncols
                        )
                        y65p = y65ps[(s, oc)]
                        dst = y65p[
                            :,
                            1 + 2 * u0 + eh : 1 + 2 * (u0 + nr) + eh : 2,
                            1 + ec : 1 + 2 * ncols + ec : 2,
                        ]
                        nc.scalar.activation(
                            dst, src, mybir.ActivationFunctionType.Copy,
                            bias=0.0,
                            scale=d_sb[:, s * NCO + oc : s * NCO + oc + 1],
                        )
                # band b evicted for both samples: column passes now.
                # Row passes lag one band so the Pool c1 op is never
                # head-of-line-waited by DVE; the last oc runs c1 on DVE, so
                # no lag there (keeps post-PE work draining incrementally).
                last = oc == NCO - 1
                for s in range(BPC):
                    emit_cols(s, oc, b)
                if last:
                    for s in range(BPC):
                        emit_rows(s, oc, b)
                elif b > 0:
                    for s in range(BPC):
                        emit_rows(s, oc, b - 1)
            if not last:
                for s in range(BPC):
                    emit_rows(s, oc, nbands - 1)
    _fix_waits(nc)
    return nc


def make_in_maps(x, w, weight, bias, affine_w, affine_b):
    x = np.asarray(x, np.float32)
    w = np.asarray(w, np.float32)
    weight = np.asarray(weight, np.float32)
    affine_w = np.asarray(affine_w, np.float32)
    affine_b = np.asarray(affine_b, np.float32)

    s = w @ affine_w.T + affine_b + 1.0  # (B, CIN)
    wsq = (weight.astype(np.float64) ** 2).sum(axis=(2, 3))  # (COUT, CIN)
    d = 1.0 / np.sqrt((s.astype(np.float64) ** 2) @ wsq.T + 1e-8)  # (B, COUT)
    d16 = (d / 16.0).astype(np.float32)

    xp = np.zeros((B, CIN, 34, 34), np.float32)
    xp[:, :, 1:33, 1:33] = x * s[:, :, None, None]
    xp_bf = xp.astype(ml_dtypes.bfloat16).reshape(B, NCI, P, 34 * 34)

    wf = weight[:, :, ::-1, ::-1]  # spatial flip
    w9 = wf.transpose(2, 3, 1, 0).reshape(9, NCI, P, COUT)
    torder = [t for taps in CLASS_TAPS for t in taps]  # class-grouped slots
    wt = np.ascontiguousarray(
        w9[torder].transpose(1, 2, 0, 3).reshape(NCI, P, 9 * COUT)
    ).astype(ml_dtypes.bfloat16)

    in_maps = []
    for core in range(NCORES):
        sl = slice(core * BPC, (core + 1) * BPC)
        dcore = d16[sl].reshape(BPC, NCO, P)
        dsc = np.ascontiguousarray(dcore.transpose(2, 0, 1).reshape(P, BPC * NCO))
        in_maps.append(
            {
                "xp": np.ascontiguousarray(xp_bf[sl]),
                "wt": wt,
                "dsc": dsc,
            }
        )
    return in_maps


LAST_RESULTS = None  # BassKernelResults of the most recent run (for test harness)


def kernel(x, w, weight, bias, affine_w, affine_b):
    global LAST_RESULTS
    in_maps = make_in_maps(x, w, weight, bias, affine_w, affine_b)
    nc = build_program()
    res = run_bass_kernel_spmd(nc, in_maps, list(range(NCORES)))
    LAST_RESULTS = res
    outs = [
        np.asarray(r["out"]).astype(np.float32).reshape(BPC, COUT, 64, 64)
        for r in res.results
    ]
    full = np.concatenate(outs, axis=0) + np.asarray(bias, np.float32).reshape(
        1, COUT, 1, 1
    )
    return np.ascontiguousarray(full, dtype=np.float32)
